# revision 1
# baseline (speedup 1.0000x reference)
"""Trainium2 Bass kernel for nn_CausalDit (sparse frame-causal DiT).

Sharding over 8 NeuronCores (SPMD, one program):
- Token space: 11 frames (6 noisy "zr" + 5 clean "xa"), 257 tokens each
  (256 patch tokens + 1 register/action token). Core c owns rows
  [32c, 32c+32) of every frame plus a replicated copy of each frame's
  leftover token. Per-core token order: [f*32+j for f, j] (352 own rows,
  frame-major), then 11 leftovers, then 1 pad = 364 rows.
- Dense compute (LN/AdaLN-mod, QKV, Wo, GEGLU FFN, gates) is token-sharded;
  attention is head-sharded (8 heads <-> 8 cores), block-sparse at frame
  granularity. An AllToAll ships Q^T/K^T/V^T (d-major, contiguous rows) to
  head owners; attention runs per frame-pair with softmax-without-max
  (denominator via a ones-column in V_aug); a second AllToAll returns
  normalized per-head outputs to token owners.
- All DRAM staging DMAs move contiguous 1456B runs (no per-element
  descriptor scatter); layout fix-ups happen on-chip via strided-AP matmul
  operands, PE transposes (V^T -> token-major V), and a single DVE gather
  for Q.
- Per-frame AdaLN scale/shift/gate tables are broadcast to token rows
  on-chip via one-hot matmuls from a tiny [12, 6, D] table (no big
  replicated table DMAs).
- Matmuls run as float32r (full PE rate for free-dim >= 256) with fp32
  PSUM accumulation.
- Host (numpy, fp32) does only tiny prep: patchify + patch matmul,
  embedding gathers, per-frame AdaLN tables, bias folds (b_k dropped -
  softmax-shift-invariant; b_v folded into b_o), and the final unpatch.
"""
import numpy as np

import concourse.bass as bass
import concourse.mybir as mybir
import concourse.tile as tile
from concourse import bacc
from concourse.bass_utils import run_bass_kernel_spmd
from concourse.masks import make_identity

# ---- model constants (hardcoded from the problem spec) ----
P2 = 2; NH = 8; NW = 4; NB = 6; D = 512; HID = 2048
HH = 32; WW = 32; C = 3; DUR = 6
DH = D // NH          # 64
NZ = DUR              # 6 zr frames
NX = DUR - 1          # 5 xa frames
NF = NZ + NX          # 11 frames
S = (HH // P2) * (WW // P2)   # 256
TPF = S + 1           # 257 tokens/frame
NCORE = 8
OWN = 32              # owned rows per frame per core
NOWN = NF * OWN       # 352 own rows
TOK = NOWN + NF       # 363 real rows per core
TOKP = TOK + 1        # padded to 364
TPQ = 258             # q columns per frame: 256 own + leftover + pad
NFP = NF + 1          # leftover rows padded to 12
VA = DH + 2           # V_aug cols: 64 v + 1 ones + 1 pad = 66
CORE_IDS = list(range(NCORE))
TOKT = [(0, 128), (128, 128), (256, 108)]   # token tiles of 364
F32 = mybir.dt.float32
F32R = mybir.dt.float32r
BF16 = mybir.dt.bfloat16
AX = mybir.AxisListType.X
ALU = mybir.AluOpType
ACTF = mybir.ActivationFunctionType

F16 = mybir.dt.float16
import os as _os_mod
PREC = _os_mod.environ.get("KERNEL_PREC", "fp32" if _os_mod.environ.get("KERNEL_FP32") else "half")
# "half" (default): attention path bf16 (exp range), FFN path fp16 (precision)
ATT_BF = PREC == "half"
FFN_F16 = PREC == "half"
USE_BF16 = ATT_BF
DTA = BF16 if ATT_BF else F32R           # attention transport dtype
DTF = F16 if FFN_F16 else F32R           # FFN transport dtype
DT = DTA
DTT = DTA                                # PE-transpose dtype (V path)


def _kv_frames(fq):
    """Global kv-frame indices for q-frame fq (zr: 0..5, xa: 6..10)."""
    if fq < NZ:
        return [fq] + [NZ + j for j in range(max(0, fq - NW), min(fq, NX))]
    return list(range(NZ, fq + 1))


def _R(ap):
    return ap.bitcast(F32R)


def _M(ap):
    """Matmul-operand view: f32 storage is bitcast to f32r; bf16 passes through."""
    return ap.bitcast(F32R) if ap.dtype == F32 else ap


def _ps32(ap):
    """Copy-source view of a PE-transpose PSUM tile (f32r reads as f32)."""
    return ap.bitcast(F32) if ap.dtype == F32R else ap


_CACHE = {}
LAST_RESULT = None


def _build(n_blocks, repeat=1):
    import os as _os
    skip_coll = bool(_os.environ.get("KERNEL_SKIP_COLL"))
    nc = bacc.Bacc("TRN2", target_bir_lowering=False, debug=False,
                   num_devices=NCORE)
    x0_e = nc.declare_dram_parameter("x0", [TOKP, D], F32, isOutput=False)
    lb_e = nc.declare_dram_parameter("lbias", [NFP, NFP], F32, isOutput=False)
    fm_e = nc.declare_dram_parameter("fmap", [NFP, 3, 128], F32R, isOutput=False)
    xout_e = nc.declare_dram_parameter("xout", [TOKP, D], F32, isOutput=True)
    ext = []
    for i in range(n_blocks):
        e = dict(
            wqkvo=nc.declare_dram_parameter(f"wqkvo{i}", [128, 16, D], DTA, isOutput=False),
            wg=nc.declare_dram_parameter(f"wg{i}", [128, 4, 2 * HID], DTF, isOutput=False),
            wf=nc.declare_dram_parameter(f"wf{i}", [128, 16, D], DTF, isOutput=False),
            bq=nc.declare_dram_parameter(f"bq{i}", [64, 1], F32, isOutput=False),
            bgl=nc.declare_dram_parameter(f"bgl{i}", [128, 32], F32, isOutput=False),
            bop=nc.declare_dram_parameter(f"bop{i}", [128, D], F32, isOutput=False),
            bff=nc.declare_dram_parameter(f"bff{i}", [128, D], F32, isOutput=False),
            tabf=nc.declare_dram_parameter(f"tabf{i}", [NFP, 6, D], F32R, isOutput=False),
        )
        ext.append(e)

    with tile.TileContext(nc) as tc:
        with (
            tc.tile_pool(name="const", bufs=1) as cpool,
            tc.tile_pool(name="blk", bufs=2) as bpool,
            tc.tile_pool(name="xp", bufs=2) as xpool,
            tc.tile_pool(name="wp", bufs=2) as wpool,
            tc.tile_pool(name="wgs", bufs=int(_os.environ.get("KERNEL_WGS", 2))) as wgpool,
            tc.tile_pool(name="act", bufs=1) as apool,
            tc.tile_pool(name="at", bufs=1) as atpool,
            tc.tile_pool(name="hp", bufs=int(_os.environ.get("KERNEL_HP", 2))) as hpool,
            tc.tile_pool(name="sc", bufs=int(_os.environ.get("KERNEL_SC", 2))) as scpool,
            tc.tile_pool(name="pt", bufs=int(_os.environ.get("KERNEL_PT", 6))) as ptpool,
            tc.tile_pool(name="psA", bufs=int(_os.environ.get("KERNEL_PSA", 2)), space="PSUM") as pspool,
            tc.tile_pool(name="psB", bufs=int(_os.environ.get("KERNEL_PSB", 4)), space="PSUM") as big3,
            tc.tile_pool(name="psT", bufs=2, space="PSUM") as pstab,
            tc.tile_pool(name="dram", bufs=1, space="DRAM") as dpool,
        ):
            ident32 = cpool.tile([128, 128], F32)
            make_identity(nc, ident32[:])
            ident = cpool.tile([128, 128], F32R)
            nc.vector.tensor_copy(ident[:], ident32[:])
            identT = cpool.tile([128, 128], DTT)
            nc.vector.tensor_copy(identT[:], ident32[:])
            eps = cpool.tile([128, 1], F32)
            nc.vector.memset(eps[:], 1e-5)
            lbias = cpool.tile([NFP, NFP], F32)
            nc.sync.dma_start(lbias[:NFP], lb_e[:])
            fmap = cpool.tile([NFP, 3, 128], F32R)
            nc.sync.dma_start(fmap[:NFP], fm_e[:])

            for rr in range(repeat):
              x = xpool.tile([128, 3, D], F32, tag="x", name=f"x_{rr}")
              nc.sync.dma_start(x[:, 0:2, :], x0_e[0:256, :].rearrange("(t r) d -> r t d", r=128))
              nc.sync.dma_start(x[:108, 2, :], x0_e[256:364, :])

              for i in range(n_blocks):
                  e = ext[i]
                  wqkvo = wpool.tile([128, 16, D], DTA, tag="wqkvo")
                  nc.sync.dma_start(wqkvo[:], e["wqkvo"][:])
                  bq_sb = bpool.tile([64, 1], F32, tag="bq")
                  nc.sync.dma_start(bq_sb[:64], e["bq"][:])
                  bgl_sb = bpool.tile([128, 32], F32, tag="bgl")
                  nc.sync.dma_start(bgl_sb[:], e["bgl"][:])
                  bop_sb = bpool.tile([128, D], F32, tag="bop")
                  nc.sync.dma_start(bop_sb[:], e["bop"][:])
                  bff_sb = bpool.tile([128, D], F32, tag="bff")
                  nc.sync.dma_start(bff_sb[:], e["bff"][:])
                  tabf = bpool.tile([NFP, 6, D], F32R, tag="tabf")
                  nc.sync.dma_start(tabf[:NFP], e["tabf"][:])

                  def tab_ps(v, tt, tabf=tabf):
                      """Broadcast per-frame table v to token rows of group
                      tt via a one-hot matmul; returns a PSUM tile."""
                      ps = pstab.tile([128, D], F32, tag="pstab")
                      nc.tensor.matmul(ps[:], fmap[:NFP, tt, :], tabf[:NFP, v, :],
                                       start=True, stop=True)
                      return ps

                  def ln_mod(srcs, v_s, v_t, xn_f32):
                      """xn = LN(srcs[tt])*tab[v_s] + tab[v_t] (fp32)."""
                      for tt, (r0, p_) in enumerate(TOKT):
                          s_ps = tab_ps(v_s, tt)
                          t_ps = tab_ps(v_t, tt)
                          xt = srcs[tt][:p_, :]
                          st6 = scpool.tile([128, 6], F32, tag="st6")
                          nc.vector.bn_stats(st6[:p_], xt)
                          mv = scpool.tile([128, 2], F32, tag="mv")
                          nc.vector.bn_aggr(mv[:p_], st6[:p_])
                          std = scpool.tile([128, 1], F32, tag="std")
                          nc.scalar.activation(std[:p_], mv[:p_, 1:2], ACTF.Sqrt,
                                               bias=eps[:p_])
                          rin = scpool.tile([128, 1], F32, tag="rin")
                          nc.vector.reciprocal(rin[:p_], std[:p_])
                          xc = scpool.tile([128, D], F32, tag="xc")
                          nc.vector.tensor_scalar(xc[:p_], xt, mv[:p_, 0:1], None,
                                                  op0=ALU.subtract)
                          tmp = scpool.tile([128, D], F32, tag="lntmp")
                          nc.vector.scalar_tensor_tensor(
                              tmp[:p_], s_ps[:p_, :], rin[:p_], xc[:p_],
                              op0=ALU.mult, op1=ALU.mult)
                          nc.vector.tensor_add(xn_f32[:p_, tt, :], tmp[:p_],
                                               t_ps[:p_, :])

                  def transpose_tok(xn_f32, xnT):
                      """xn [128,3,D] f32 -> xnT [128,4,TOKP] (d-major)."""
                      for tt, (r0, p_) in enumerate(TOKT):
                          pst = big3.tile([128, 512], F32R, tag="ps3")
                          for kd in range(4):
                              nc.tensor.transpose(pst[:, 128 * kd:128 * kd + p_],
                                                  xn_f32[:p_, tt, kd * 128:(kd + 1) * 128],
                                                  ident[:p_, :p_])
                          nc.vector.tensor_copy(
                              xnT[:, :, r0:r0 + p_],
                              pst[:].rearrange("p (k c) -> p k c", k=4)[:, :, :p_].bitcast(F32))

                  # ---- mod1 ----
                  xn1 = apool.tile([128, 3, D], F32R, tag="xn1")
                  ln_mod([x[:, 0, :], x[:, 1, :], x[:, 2, :]], 0, 1, xn1)
                  xnT = apool.tile([128, 4, TOKP], DT, tag="xnT")
                  transpose_tok(xn1, xnT)

                  # ---- QKV^T (d-major) ----
                  qkvt = apool.tile([128, 3, 4, TOKP], DT, tag="qkvt")
                  qt_sb = qkvt[:, 0]
                  kt_sb = qkvt[:, 1]
                  vt_sb = qkvt[:, 2]
                  for m in range(4):
                      psq = pspool.tile([128, TOKP], F32, tag="psbig")
                      for k in range(4):
                          nc.tensor.matmul(psq[:], wqkvo[:, k, m * 128:(m + 1) * 128],
                                           xnT[:, k, :], start=(k == 0), stop=(k == 3))
                      nc.vector.tensor_copy(qt_sb[:, m, :], psq[:])
                      psk = pspool.tile([128, TOKP], F32, tag="psbig")
                      for k in range(4):
                          nc.tensor.matmul(psk[:], wqkvo[:, 4 + k, m * 128:(m + 1) * 128],
                                           xnT[:, k, :], start=(k == 0), stop=(k == 3))
                      nc.vector.tensor_copy(kt_sb[:, m, :], psk[:])
                      psv = pspool.tile([128, TOKP], F32, tag="psbig")
                      for k in range(4):
                          nc.tensor.matmul(psv[:], wqkvo[:, 8 + k, m * 128:(m + 1) * 128],
                                           xnT[:, k, :], start=(k == 0), stop=(k == 3))
                      nc.vector.tensor_copy(vt_sb[:, m, :], psv[:])

                  # ---- A2A fwd: chunk d = [64, 3, TOKP] = Q^T|K^T|V^T of head d ----
                  # gate1 products are A2A-independent: compute them here so the
                  # DVE/PE streams have work during the collective round-trip
                  g1x_t = []
                  for tt, (r0, p_) in enumerate(TOKT):
                      g1_ps = tab_ps(2, tt)
                      g1x = apool.tile([128, D], F32, tag=f"g1x_{tt}")
                      nc.vector.tensor_mul(g1x[:p_], xn1[:p_, tt, :].bitcast(F32),
                                           g1_ps[:p_, :])
                      g1x_t.append(g1x)

                  a2a_s = dpool.tile([NCORE, 64, 3, TOKP], DT, tag=f"a2as{rr}_{i}")
                  a2a_r = dpool.tile([NCORE, 64, 3, TOKP], DT, tag=f"a2ar{rr}_{i}")
                  nc.sync.dma_start(
                      a2a_s.rearrange("(c p2) r t x -> (p2 r) t c x", c=4),
                      qkvt[:])
                  if skip_coll:
                      nc.sync.dma_start(a2a_r[:], a2a_s[:])
                  else:
                      nc.gpsimd.collective_compute(
                          "AllToAll", ALU.bypass, replica_groups=[CORE_IDS],
                          ins=[a2a_s.opt()], outs=[a2a_r.opt()])

                  # ---- load this head's Q^T/K^T/V^T (src-major) ----
                  qkvs = atpool.tile([64, 3, NCORE, TOKP], DT, tag="qkvs")
                  nc.sync.dma_start(qkvs[:], a2a_r.rearrange("s r t x -> r t s x"))
                  qs = qkvs[:, 0]
                  ks = qkvs[:, 1]
                  vs = qkvs[:, 2]

                  # ---- assemble Q frame-major; V^T -> token-major V_aug ----
                  qtA = atpool.tile([64, NF, TPQ], DT, tag="qtA")
                  nc.vector.tensor_scalar(
                      qtA[:, :, 0:S].rearrange("p f (s j) -> p f s j", s=NCORE),
                      qs[:, :, 0:NOWN].rearrange("p s (f j) -> p f s j", f=NF),
                      bq_sb[:64, 0:1], None, op0=ALU.add)
                  nc.vector.tensor_scalar(qtA[:, :, S], qs[:, 7, NOWN:TOK],
                                          bq_sb[:64, 0:1], None, op0=ALU.add)
                  nc.vector.memset(qtA[:, :, S + 1], 0.0)

                  ktA = atpool.tile([64, NF, S], DT, tag="ktA")
                  nc.vector.tensor_copy(
                      ktA[:].rearrange("p f (s j) -> p f s j", s=NCORE),
                      ks[:, :, 0:NOWN].rearrange("p s (f j) -> p f s j", f=NF))
                  ktl = atpool.tile([64, NFP], DT, tag="ktl")
                  nc.vector.tensor_copy(ktl[:64, :], ks[:, 7, NOWN:TOKP])
                  vtA = atpool.tile([64, NF, S], DT, tag="vtA")
                  nc.vector.tensor_copy(
                      vtA[:].rearrange("p f (s j) -> p f s j", s=NCORE),
                      vs[:, :, 0:NOWN].rearrange("p s (f j) -> p f s j", f=NF))

                  v_a = atpool.tile([128, 2 * NF, VA], DT, tag="va")
                  nc.vector.memset(v_a[:, :, DH], 1.0)
                  nc.vector.memset(v_a[:, :, DH + 1], 0.0)
                  for f in range(NF):
                      pst = big3.tile([128, 128], DTT, tag="ps3")
                      for t2 in range(2):
                          nc.tensor.transpose(
                              pst[:, 64 * t2:64 * t2 + 64],
                              vtA[:, f, 128 * t2:128 * (t2 + 1)],
                              identT[:64, :64])
                      nc.vector.tensor_copy(
                          v_a[:, 2 * f:2 * f + 2, 0:DH],
                          _ps32(pst[:].rearrange("p (t c) -> p t c", t=2)))
                  v_l = atpool.tile([NFP, VA], DT, tag="vl")
                  nc.vector.memset(v_l[:NFP, DH:DH + 1], 1.0)
                  nc.vector.memset(v_l[:NFP, DH + 1:DH + 2], 0.0)
                  pstl = big3.tile([NFP, 64], DTT, tag="ps3")
                  nc.tensor.transpose(pstl[:NFP, :64], _M(vs[:, 7, NOWN:TOKP]),
                                      identT[:64, :64])
                  nc.vector.tensor_copy(v_l[:NFP, 0:DH], _ps32(pstl[:NFP, :64]))

                  # ---- attention (this head, all 11 q-frames) ----
                  otA = atpool.tile([64, NF, TPQ], DT, tag="qkvs")
                  nfq = NZ if i == n_blocks - 1 else NF
                  for fq in range(nfq):
                      kvf = _kv_frames(fq)
                      n_main = 2 * len(kvf)
                      ps_o = pspool.tile([VA, TPQ], F32, tag="psbig")
                      ps_l = big3.tile([NFP, TPQ], F32, tag="ps3")
                      nc.tensor.matmul(ps_l[:NFP], _M(ktl[:64, :]),
                                       _M(qtA[:, fq, :]), start=True, stop=True)
                      pl = ptpool.tile([NFP, TPQ], DTT, tag="pt")
                      nc.scalar.activation(pl[:NFP, :], ps_l[:NFP, :], ACTF.Exp,
                                           scale=0.125, bias=lbias[:NFP, fq:fq + 1])
                      nc.tensor.matmul(ps_o[:], _M(v_l[:NFP, :]), pl[:NFP],
                                       start=True, stop=False)
                      av_i = 0
                      for fi in kvf:
                          for t2 in range(2):
                              ps_s = big3.tile([128, TPQ], F32, tag="ps3")
                              nc.tensor.matmul(
                                  ps_s[:],
                                  ktA[:, fi, 128 * t2:128 * (t2 + 1)],
                                  _M(qtA[:, fq, :]), start=True, stop=True)
                              pt = ptpool.tile([128, TPQ], DTT, tag="pt")
                              nc.scalar.activation(pt[:], ps_s[:], ACTF.Exp, scale=0.125)
                              nc.tensor.matmul(ps_o[:], _M(v_a[:, 2 * fi + t2, :]), pt[:],
                                               start=False, stop=(av_i == n_main - 1))
                              av_i += 1
                      rc = scpool.tile([1, TPQ], F32, tag="rc")
                      nc.vector.reciprocal(rc[0:1], ps_o[DH:DH + 1, :])
                      bc = scpool.tile([64, TPQ], F32, tag="rc")
                      nc.gpsimd.partition_broadcast(bc[:64], rc[0:1, :])
                      nc.vector.tensor_mul(otA[:, fq, :], ps_o[0:DH, :], bc[:64])

                  # ---- A2A back: chunk d = my head's outputs for d's tokens ----
                  bk_send = atpool.tile([64, NCORE, TOKP], DT, tag="bks")
                  nc.vector.tensor_copy(
                      bk_send[:, :, 0:NOWN].rearrange("p d (f j) -> p d f j", f=NF),
                      otA[:, :, 0:S].rearrange("p f (d j) -> p d f j", d=NCORE))
                  nc.vector.tensor_copy(
                      bk_send[:, :, NOWN:TOK],
                      otA[:, None, :, S].broadcast_to([64, NCORE, NF]))
                  nc.vector.memset(bk_send[:, :, TOK], 0.0)
                  bk_s = dpool.tile([NCORE, 64, TOKP], DT, tag=f"bks{rr}_{i}")
                  bk_r = dpool.tile([NCORE, 64, TOKP], DT, tag=f"bkr{rr}_{i}")
                  nc.sync.dma_start(bk_s.rearrange("d r x -> r d x"), bk_send[:])
                  if skip_coll:
                      nc.sync.dma_start(bk_r[:], bk_s[:])
                  else:
                      nc.gpsimd.collective_compute(
                          "AllToAll", ALU.bypass, replica_groups=[CORE_IDS],
                          ins=[bk_s.opt()], outs=[bk_r.opt()])
                  xaT = apool.tile([128, 4, TOKP], DT, tag="qt")
                  nc.sync.dma_start(xaT[:],
                                    bk_r.rearrange("(c p2) r x -> (p2 r) c x", c=4))

                  # ---- Wo + gate1*xn1 + bo' ----
                  x2g = []
                  for tt, (r0, p_) in enumerate(TOKT):
                      x2t = apool.tile([128, D], F32, tag=f"x2_{tt}")
                      pso = pspool.tile([128, D], F32, tag="psbig")
                      for k in range(4):
                          nc.tensor.matmul(pso[:p_], _M(xaT[:, k, r0:r0 + p_]),
                                           wqkvo[:, 12 + k, :], start=(k == 0), stop=(k == 3))
                      tmp = scpool.tile([128, D], F32, tag="tmpD")
                      nc.vector.tensor_add(tmp[:p_], pso[:p_], bop_sb[:p_])
                      nc.vector.tensor_add(x2t[:p_, :], tmp[:p_], g1x_t[tt][:p_, :])
                      x2g.append(x2t)

                  # ---- mod2 ----
                  xn2 = apool.tile([128, 3, D], F32R, tag="xn1")
                  ln_mod(x2g, 3, 4, xn2)
                  xn2T = apool.tile([128, 4, TOKP], DTF, tag="xnT")
                  transpose_tok(xn2, xn2T)

                  # ---- GEGLU + FF out, streamed in 8 weight pieces ----
                  psf = [big3.tile([128, D], F32, tag="ps3", name=f"psf{rr}_{i}_{tt}")
                         for tt in range(3)]
                  for p in range(8):
                      wgp = wgpool.tile([128, 4, 512], DTF, tag="wgp")
                      nc.sync.dma_start(wgp[:], e["wg"][:, :, 512 * p:512 * (p + 1)])
                      wfp = wgpool.tile([128, 2, D], DTF, tag="wfp")
                      nc.sync.dma_start(wfp[:], e["wf"][:, 2 * p:2 * (p + 1), :])
                      for j in range(2):
                          mm = 2 * p + j
                          # last block: xa tokens are dead past attention; compute
                          # only zr columns (0:192 own rows, 352:360 leftovers)
                          tok_rs = ((0, 192), (NOWN, NOWN + 8)) if i == n_blocks - 1 \
                              else ((0, TOKP),)
                          psa = pspool.tile([128, TOKP], F32, tag="psbig")
                          for c0, c1 in tok_rs:
                              for k in range(4):
                                  nc.tensor.matmul(psa[:, c0:c1], wgp[:, k, 256 * j:256 * j + 128],
                                                   xn2T[:, k, c0:c1], start=(k == 0), stop=(k == 3))
                          # pstab's banks are idle through the GEGLU body; using
                          # them for psg doubles the a/g matmul pipeline depth
                          psg = pstab.tile([128, TOKP], F32, tag="pstab")
                          for c0, c1 in tok_rs:
                              for k in range(4):
                                  nc.tensor.matmul(psg[:, c0:c1], wgp[:, k, 256 * j + 128:256 * j + 256],
                                                   xn2T[:, k, c0:c1], start=(k == 0), stop=(k == 3))
                          gel = scpool.tile([128, TOKP], F32, tag="gel")
                          hp = hpool.tile([128, TOKP], DTF, tag="hp")
                          for c0, c1 in tok_rs:
                              nc.scalar.activation(gel[:, c0:c1], psg[:, c0:c1], ACTF.Gelu,
                                                   bias=bgl_sb[:, 2 * mm + 1:2 * mm + 2])
                              nc.vector.scalar_tensor_tensor(hp[:, c0:c1], psa[:, c0:c1],
                                                             bgl_sb[:, 2 * mm:2 * mm + 1], gel[:, c0:c1],
                                                             op0=ALU.add, op1=ALU.mult)
                          for tt, (r0, p_) in enumerate(TOKT):
                              nc.tensor.matmul(psf[tt][:p_], _M(hp[:, r0:r0 + p_]),
                                               wfp[:, j, :],
                                               start=(mm == 0), stop=(mm == 15))

                  x_new = xpool.tile([128, 3, D], F32, tag="x")
                  for tt, (r0, p_) in enumerate(TOKT):
                      g2_ps = tab_ps(5, tt)
                      tmp = scpool.tile([128, D], F32, tag="tmpD")
                      nc.vector.tensor_add(tmp[:p_], psf[tt][:p_], bff_sb[:p_])
                      nc.vector.tensor_mul(x_new[:p_, tt, :], tmp[:p_], g2_ps[:p_, :])
                  x = x_new
                  if _os.environ.get("KERNEL_BLOCK_BARRIER"):
                      # optional scheduling barrier between blocks (collective
                      # ordering is data-enforced; barrier-free validated on HW)
                      tc.strict_bb_all_engine_barrier()

            nc.sync.dma_start(xout_e[0:256, :].rearrange("(t r) d -> r t d", r=128),
                              x[:, 0:2, :])
            nc.sync.dma_start(xout_e[256:364, :], x[:108, 2, :])
    nc.compile()
    return nc


# ----------------------------------------------------------------------
# host side
# ----------------------------------------------------------------------
def _silu(x):
    return x / (1.0 + np.exp(-x))


def _frame_of():
    """frame index of each per-core token slot (12 = pad/zero row)."""
    fr = np.full(TOKP, NF, np.int64)
    fr[:NOWN] = np.arange(NOWN) // OWN
    fr[NOWN:TOK] = np.arange(NF)
    return fr


def _host_prep(inputs, n_blocks):
    f32 = np.float32
    z = np.asarray(inputs['z'], f32)
    frames = np.asarray(inputs['frames'], f32)
    actions = np.asarray(inputs['actions'])
    ts = np.asarray(inputs['ts'])

    def patch(xx):
        b, dur, c, h, w = xx.shape
        xx = xx.reshape(b, dur, c, h // P2, P2, w // P2, P2)
        xx = xx.transpose(0, 1, 3, 5, 2, 4, 6).reshape(b, dur, (h // P2) * (w // P2), c * P2 * P2)
        return xx @ np.asarray(inputs['W_patch'], f32) + np.asarray(inputs['b_patch'], f32)

    pe = np.asarray(inputs['pe_grid'], f32)
    zt = patch(z)[0] + pe[None]
    xt = patch(frames)[0] + pe[None]
    reg = np.asarray(inputs['registers'], f32)
    aemb = np.asarray(inputs['action_emb'], f32)
    temb = np.asarray(inputs['time_emb'], f32)
    a = aemb[actions[0]]

    ft = np.zeros((NF, TPF, D), f32)
    for f in range(NZ):
        ft[f, :S] = zt[f]
        ft[f, S] = reg[0]
    for f in range(NX):
        ft[NZ + f, :S] = xt[f]
        ft[NZ + f, S] = a[f]

    cond = np.zeros((NF, D), f32)
    for f in range(NZ):
        cond[f] = temb[ts[0, f]]
    for f in range(NX):
        cond[NZ + f] = temb[0]
    sc = _silu(cond)

    blocks = []
    for i in range(n_blocks):
        m1 = sc @ np.asarray(inputs['W_mod1'][i], f32) + np.asarray(inputs['b_mod1'][i], f32)
        s1, t1 = np.split(m1, 2, -1)
        m2 = sc @ np.asarray(inputs['W_mod2'][i], f32) + np.asarray(inputs['b_mod2'][i], f32)
        s2, t2 = np.split(m2, 2, -1)
        g1 = cond @ np.asarray(inputs['W_g1'][i], f32) + np.asarray(inputs['b_g1'][i], f32)
        g2 = cond @ np.asarray(inputs['W_g2'][i], f32) + np.asarray(inputs['b_g2'][i], f32)
        bo_p = (np.asarray(inputs['b_o'][i], f32)
                + np.asarray(inputs['b_v'][i], f32) @ np.asarray(inputs['W_o'][i], f32))
        tabf = np.zeros((NFP, 6, D), f32)
        tabf[:NF] = np.stack([1.0 + s1, t1, g1, 1.0 + s2, t2, g2], 1)

        def chunk(w, kparts):
            K, N = w.shape
            return np.ascontiguousarray(
                np.asarray(w, f32).reshape(kparts, 128, N).swapaxes(0, 1))

        wq = chunk(np.asarray(inputs['W_q'][i]), 4)
        wk = chunk(np.asarray(inputs['W_k'][i]), 4)
        wv = chunk(np.asarray(inputs['W_v'][i]), 4)
        wo = chunk(np.asarray(inputs['W_o'][i]), 4)
        wqkvo = np.concatenate([wq, wk, wv, wo], 1)

        # interleave a/g columns of W_geglu so each 256-col group is (a_mm|g_mm)
        wg = chunk(np.asarray(inputs['W_geglu'][i]), 4)        # [128, 4, 4096]
        wg4 = wg.reshape(128, 4, 2, 16, 128)                   # [., ., a/g, mm, col]
        wg_i = np.ascontiguousarray(
            wg4.transpose(0, 1, 3, 2, 4).reshape(128, 4, 4096))
        bg = np.asarray(inputs['b_geglu'][i], f32).reshape(2, 16, 128)
        bgl = np.ascontiguousarray(
            bg.transpose(2, 1, 0).reshape(128, 32))            # [128, 32] cols (2mm, 2mm+1)

        wf_i = chunk(np.asarray(inputs['W_ffout'][i]), 16)
        if ATT_BF:
            import ml_dtypes
            wqkvo = wqkvo.astype(ml_dtypes.bfloat16)
        if FFN_F16:
            wg_i = wg_i.astype(np.float16)
            wf_i = wf_i.astype(np.float16)
        blocks.append(dict(
            wqkvo=wqkvo,
            wg=wg_i,
            wf=wf_i,
            bq=np.asarray(inputs['b_q'][i], f32),   # sliced per core below
            bgl=bgl,
            bop=np.tile(bo_p[None], (128, 1)).astype(f32),
            bff=np.tile(np.asarray(inputs['b_ffout'][i], f32)[None], (128, 1)),
            tabf=tabf,
        ))
    return ft, blocks


def kernel(**inputs):
    import os
    n_blocks = int(os.environ.get("KERNEL_NBLOCKS", NB))
    ft, blocks = _host_prep(inputs, n_blocks)

    lb = np.full((NFP, NFP), -30.0, np.float32)
    for fq in range(NF):
        for kf in _kv_frames(fq):
            lb[kf, fq] = 0.0

    fr = _frame_of()
    fmap = np.zeros((NFP, 3, 128), np.float32)
    for g in range(3):
        for p in range(128):
            t = g * 128 + p
            if t < TOKP and fr[t] < NF:
                fmap[fr[t], g, p] = 1.0

    in_maps = []
    for c in range(NCORE):
        x0p = np.zeros((TOKP, D), np.float32)
        for f in range(NF):
            x0p[f * OWN:(f + 1) * OWN] = ft[f, OWN * c:OWN * (c + 1)]
            x0p[NOWN + f] = ft[f, S]
        m = {"x0": x0p, "lbias": lb, "fmap": fmap}
        for i in range(n_blocks):
            for k, v in blocks[i].items():
                if k == "bq":
                    v = np.ascontiguousarray(v[64 * c:64 * (c + 1)].reshape(64, 1))
                m[f"{k}{i}"] = v
        in_maps.append(m)

    repeat = int(os.environ.get("KERNEL_REPEAT", 1))
    key = (n_blocks, repeat)
    if key not in _CACHE:
        _CACHE[key] = _build(n_blocks, repeat)
    nc = _CACHE[key]
    trace = bool(os.environ.get("KERNEL_TRACE"))
    res = run_bass_kernel_spmd(nc, in_maps, CORE_IDS, trace=trace)
    global LAST_RESULT
    LAST_RESULT = res

    out = np.zeros((NF, TPF, D), np.float32)
    for c in range(NCORE):
        xo = res.results[c]["xout"]
        for f in range(NF):
            out[f, OWN * c:OWN * (c + 1)] = xo[f * OWN:(f + 1) * OWN]
    x0 = res.results[0]["xout"]
    for f in range(NF):
        out[f, S] = x0[NOWN + f]

    f32 = np.float32
    zr = out[:NZ, :S]
    y = zr @ np.asarray(inputs['W_unpatch'], f32) + np.asarray(inputs['b_unpatch'], f32)
    y = y.reshape(1, NZ, HH // P2, WW // P2, C, P2, P2)
    y = y.transpose(0, 1, 4, 2, 5, 3, 6).reshape(1, NZ, C, HH, WW)
    return np.ascontiguousarray(y.astype(np.float32))



# revision 12
# speedup vs baseline: 1.0791x; 1.0791x over previous
"""Trainium2 Bass kernel for nn_CausalDit (sparse frame-causal DiT).

Sharding over 8 NeuronCores (SPMD, one program):
- Token space: 11 frames (6 noisy "zr" + 5 clean "xa"), 257 tokens each
  (256 patch tokens + 1 register/action token). Core c owns rows
  [32c, 32c+32) of every frame plus a replicated copy of each frame's
  leftover token. Per-core token order: [f*32+j for f, j] (352 own rows,
  frame-major), then 11 leftovers, then 1 pad = 364 rows.
- Dense compute (LN/AdaLN-mod, QKV, Wo, GEGLU FFN, gates) is token-sharded;
  attention is head-sharded (8 heads <-> 8 cores), block-sparse at frame
  granularity. An AllToAll ships Q^T/K^T/V^T (d-major, contiguous rows) to
  head owners; attention runs per frame-pair with softmax-without-max
  (denominator via a ones-column in V_aug); a second AllToAll returns
  normalized per-head outputs to token owners.
- All DRAM staging DMAs move contiguous 1456B runs (no per-element
  descriptor scatter); layout fix-ups happen on-chip via strided-AP matmul
  operands, PE transposes (V^T -> token-major V), and a single DVE gather
  for Q.
- Per-frame AdaLN scale/shift/gate tables are broadcast to token rows
  on-chip via one-hot matmuls from a tiny [12, 6, D] table (no big
  replicated table DMAs).
- Matmuls run as float32r (full PE rate for free-dim >= 256) with fp32
  PSUM accumulation.
- Host (numpy, fp32) does only tiny prep: patchify + patch matmul,
  embedding gathers, per-frame AdaLN tables, bias folds (b_k dropped -
  softmax-shift-invariant; b_v folded into b_o), and the final unpatch.
"""
import numpy as np

import concourse.bass as bass
import concourse.mybir as mybir
import concourse.tile as tile
from concourse import bacc
from concourse.bass_utils import run_bass_kernel_spmd
from concourse.masks import make_identity

# ---- model constants (hardcoded from the problem spec) ----
P2 = 2; NH = 8; NW = 4; NB = 6; D = 512; HID = 2048
HH = 32; WW = 32; C = 3; DUR = 6
DH = D // NH          # 64
NZ = DUR              # 6 zr frames
NX = DUR - 1          # 5 xa frames
NF = NZ + NX          # 11 frames
S = (HH // P2) * (WW // P2)   # 256
TPF = S + 1           # 257 tokens/frame
NCORE = 8
OWN = 32              # owned rows per frame per core
NOWN = NF * OWN       # 352 own rows
TOK = NOWN + NF       # 363 real rows per core
TOKP = TOK + 1        # padded to 364
TPQ = 258             # q columns per frame: 256 own + leftover + pad
NFP = NF + 1          # leftover rows padded to 12
VA = DH + 2           # V_aug cols: 64 v + 1 ones + 1 pad = 66
CORE_IDS = list(range(NCORE))
TOKT = [(0, 128), (128, 128), (256, 108)]   # token tiles of 364
F32 = mybir.dt.float32
F32R = mybir.dt.float32r
BF16 = mybir.dt.bfloat16
AX = mybir.AxisListType.X
ALU = mybir.AluOpType
ACTF = mybir.ActivationFunctionType

F16 = mybir.dt.float16
import os as _os_mod
PREC = _os_mod.environ.get("KERNEL_PREC", "fp32" if _os_mod.environ.get("KERNEL_FP32") else "half")
# "half" (default): attention path bf16 (exp range), FFN path fp16 (precision)
ATT_BF = PREC == "half"
FFN_F16 = PREC == "half"
USE_BF16 = ATT_BF
DTA = BF16 if ATT_BF else F32R           # attention transport dtype
DTF = F16 if FFN_F16 else F32R           # FFN transport dtype
DT = DTA
DTT = DTA                                # PE-transpose dtype (V path)


def _kv_frames(fq):
    """Global kv-frame indices for q-frame fq (zr: 0..5, xa: 6..10)."""
    if fq < NZ:
        return [fq] + [NZ + j for j in range(max(0, fq - NW), min(fq, NX))]
    return list(range(NZ, fq + 1))


def _R(ap):
    return ap.bitcast(F32R)


def _M(ap):
    """Matmul-operand view: f32 storage is bitcast to f32r; bf16 passes through."""
    return ap.bitcast(F32R) if ap.dtype == F32 else ap


def _ps32(ap):
    """Copy-source view of a PE-transpose PSUM tile (f32r reads as f32)."""
    return ap.bitcast(F32) if ap.dtype == F32R else ap


_CACHE = {}
LAST_RESULT = None


def _build(n_blocks, repeat=1):
    import os as _os
    skip_coll = bool(_os.environ.get("KERNEL_SKIP_COLL"))
    nc = bacc.Bacc("TRN2", target_bir_lowering=False, debug=False,
                   num_devices=NCORE)
    x0_e = nc.declare_dram_parameter("x0", [TOKP, D], F32, isOutput=False)
    lb_e = nc.declare_dram_parameter("lbias", [NFP, NFP], F32, isOutput=False)
    xout_e = nc.declare_dram_parameter("xout", [TOKP, D], F32, isOutput=True)
    ext = []
    for i in range(n_blocks):
        e = dict(
            wqkvo=nc.declare_dram_parameter(f"wqkvo{i}", [128, 16, D], DTA, isOutput=False),
            wg=nc.declare_dram_parameter(f"wg{i}", [128, 4, 2 * HID], DTF, isOutput=False),
            wf=nc.declare_dram_parameter(f"wf{i}", [128, 16, D], DTF, isOutput=False),
            bq=nc.declare_dram_parameter(f"bq{i}", [64, 1], F32, isOutput=False),
            bgl=nc.declare_dram_parameter(f"bgl{i}", [128, 32], F32, isOutput=False),
            bop=nc.declare_dram_parameter(f"bop{i}", [128, D], F32, isOutput=False),
            bff=nc.declare_dram_parameter(f"bff{i}", [128, D], F32, isOutput=False),
            # token-broadcast AdaLN tables, host-precomputed: [p, tt*6+v, d]
            tabb=nc.declare_dram_parameter(f"tabb{i}", [128, 18, D], F16, isOutput=False),
        )
        ext.append(e)

    with tile.TileContext(nc) as tc:
        with (
            tc.tile_pool(name="const", bufs=1) as cpool,
            tc.tile_pool(name="blk", bufs=2) as bpool,
            tc.tile_pool(name="xp", bufs=2) as xpool,
            tc.tile_pool(name="wp", bufs=2) as wpool,
            tc.tile_pool(name="wgs", bufs=int(_os.environ.get("KERNEL_WGS", 2))) as wgpool,
            tc.tile_pool(name="act", bufs=1) as apool,
            tc.tile_pool(name="at", bufs=1) as atpool,
            tc.tile_pool(name="hp", bufs=int(_os.environ.get("KERNEL_HP", 2))) as hpool,
            tc.tile_pool(name="sc", bufs=int(_os.environ.get("KERNEL_SC", 2))) as scpool,
            tc.tile_pool(name="pt", bufs=int(_os.environ.get("KERNEL_PT", 6))) as ptpool,
            tc.tile_pool(name="psA", bufs=int(_os.environ.get("KERNEL_PSA", 2)), space="PSUM") as pspool,
            tc.tile_pool(name="psB", bufs=int(_os.environ.get("KERNEL_PSB", 4)), space="PSUM") as big3,
            tc.tile_pool(name="psT", bufs=2, space="PSUM") as pstab,
            tc.tile_pool(name="dram", bufs=1, space="DRAM") as dpool,
        ):
            # warmup collective: absorbs the ~25us first-call ncfw cost and
            # the cross-core entry barrier while the PE does block-0 prep
            warm_sb = cpool.tile([NCORE, 16], DT)
            nc.vector.memset(warm_sb[:NCORE], 0.0)
            warm_s = dpool.tile([NCORE, 16], DT, tag="warms")
            warm_r = dpool.tile([NCORE, 16], DT, tag="warmr")
            nc.sync.dma_start(warm_s[:], warm_sb[:NCORE])
            if not skip_coll:
                nc.gpsimd.collective_compute(
                    "AllToAll", ALU.bypass, replica_groups=[CORE_IDS],
                    ins=[warm_s.opt()], outs=[warm_r.opt()])

            ident32 = cpool.tile([128, 128], F32)
            make_identity(nc, ident32[:])
            ident = cpool.tile([128, 128], F32R)
            nc.vector.tensor_copy(ident[:], ident32[:])
            identT = cpool.tile([128, 128], DTT)
            nc.vector.tensor_copy(identT[:], ident32[:])
            eps = cpool.tile([128, 1], F32)
            nc.vector.memset(eps[:], 1e-5)
            lbias = cpool.tile([NFP, NFP], F32)
            nc.sync.dma_start(lbias[:NFP], lb_e[:])

            for rr in range(repeat):
              x = xpool.tile([128, 3, D], F32, tag="x", name=f"x_{rr}")
              nc.sync.dma_start(x[:, 0:2, :], x0_e[0:256, :].rearrange("(t r) d -> r t d", r=128))
              nc.sync.dma_start(x[:108, 2, :], x0_e[256:364, :])

              for i in range(n_blocks):
                  e = ext[i]
                  wqkvo = wpool.tile([128, 16, D], DTA, tag="wqkvo")
                  nc.sync.dma_start(wqkvo[:], e["wqkvo"][:])
                  bq_sb = bpool.tile([64, 1], F32, tag="bq")
                  nc.sync.dma_start(bq_sb[:64], e["bq"][:])
                  bgl_sb = bpool.tile([128, 32], F32, tag="bgl")
                  nc.sync.dma_start(bgl_sb[:], e["bgl"][:])
                  bop_sb = bpool.tile([128, D], F32, tag="bop")
                  nc.sync.dma_start(bop_sb[:], e["bop"][:])
                  bff_sb = bpool.tile([128, D], F32, tag="bff")
                  nc.sync.dma_start(bff_sb[:], e["bff"][:])
                  tabb = bpool.tile([128, 18, D], F16, tag="tabb")
                  nc.sync.dma_start(tabb[:], e["tabb"][:])

                  def ln_mod(srcs, v_s, v_t, xn_f32, tts=None):
                      """xn = LN(srcs[tt])*tab[v_s] + tab[v_t] (fp32)."""
                      for tt, (r0, p_) in (tts if tts is not None
                                           else enumerate(TOKT)):
                          s_tb = tabb[:, tt * 6 + v_s, :]
                          t_tb = tabb[:, tt * 6 + v_t, :]
                          xt = srcs[tt][:p_, :]
                          st6 = scpool.tile([128, 6], F32, tag="st6")
                          nc.vector.bn_stats(st6[:p_], xt)
                          mv = scpool.tile([128, 2], F32, tag="mv")
                          nc.vector.bn_aggr(mv[:p_], st6[:p_])
                          std = scpool.tile([128, 1], F32, tag="std")
                          nc.scalar.activation(std[:p_], mv[:p_, 1:2], ACTF.Sqrt,
                                               bias=eps[:p_])
                          rin = scpool.tile([128, 1], F32, tag="rin")
                          nc.vector.reciprocal(rin[:p_], std[:p_])
                          xc = scpool.tile([128, D], F32, tag="xc")
                          nc.vector.tensor_scalar(xc[:p_], xt, mv[:p_, 0:1], None,
                                                  op0=ALU.subtract)
                          tmp = scpool.tile([128, D], F32, tag="lntmp")
                          nc.vector.scalar_tensor_tensor(
                              tmp[:p_], s_tb[:p_, :], rin[:p_], xc[:p_],
                              op0=ALU.mult, op1=ALU.mult)
                          nc.vector.tensor_add(xn_f32[:p_, tt, :], tmp[:p_],
                                               t_tb[:p_, :])

                  def transpose_tok(xn_f32, xnT, tts=None):
                      """xn [128,3,D] f32 -> xnT [128,4,TOKP] (d-major)."""
                      for tt, (r0, p_) in (tts if tts is not None
                                           else enumerate(TOKT)):
                          pst = big3.tile([128, 512], F32R, tag="ps3")
                          for kd in range(4):
                              nc.tensor.transpose(pst[:, 128 * kd:128 * kd + p_],
                                                  xn_f32[:p_, tt, kd * 128:(kd + 1) * 128],
                                                  ident[:p_, :p_])
                          nc.vector.tensor_copy(
                              xnT[:, :, r0:r0 + p_],
                              pst[:].rearrange("p (k c) -> p k c", k=4)[:, :, :p_].bitcast(F32))

                  # ---- mod1 ----
                  xn1 = apool.tile([128, 3, D], F32R, tag="xn1")
                  ln_mod([x[:, 0, :], x[:, 1, :], x[:, 2, :]], 0, 1, xn1)
                  xnT = apool.tile([128, 4, TOKP], DT, tag="xnT")
                  transpose_tok(xn1, xnT)

                  # ---- QKV^T (d-major); each (t,m) chunk is DMAed to the
                  # A2A send buffer as soon as its PSUM->SBUF cast lands, so
                  # the collective can trigger right after the last matmul
                  a2a_s = dpool.tile([NCORE, 64, 3, TOKP], DT, tag=f"a2as{rr}_{i}")
                  a2a_r = dpool.tile([NCORE, 64, 3, TOKP], DT, tag=f"a2ar{rr}_{i}")
                  qkvt = apool.tile([128, 3, 4, TOKP], DT, tag="qkvt")
                  for m in range(4):
                      for t in range(3):
                          pst_ = pspool.tile([128, TOKP], F32, tag="psbig")
                          for k in range(4):
                              nc.tensor.matmul(pst_[:], wqkvo[:, 4 * t + k, m * 128:(m + 1) * 128],
                                               xnT[:, k, :], start=(k == 0), stop=(k == 3))
                          nc.vector.tensor_copy(qkvt[:, t, m, :], pst_[:])
                          nc.sync.dma_start(
                              a2a_s[2 * m:2 * m + 2, :, t, :].rearrange("d r x -> (d r) x"),
                              qkvt[:, t, m, :])

                  # ---- A2A fwd: chunk d = [64, 3, TOKP] = Q^T|K^T|V^T of head d ----
                  # gate1 products are A2A-independent: compute them here so the
                  # DVE/PE streams have work during the collective round-trip
                  g1x_t = []
                  for tt, (r0, p_) in enumerate(TOKT):
                      g1x = apool.tile([128, D], F32, tag=f"g1x_{tt}")
                      nc.vector.tensor_mul(g1x[:p_], xn1[:p_, tt, :].bitcast(F32),
                                           tabb[:p_, tt * 6 + 2, :])
                      g1x_t.append(g1x)

                  if skip_coll:
                      nc.sync.dma_start(a2a_r[:], a2a_s[:])
                  else:
                      nc.gpsimd.collective_compute(
                          "AllToAll", ALU.bypass, replica_groups=[CORE_IDS],
                          ins=[a2a_s.opt()], outs=[a2a_r.opt()])

                  # ---- load this head's Q^T/K^T/V^T (src-major); V first so
                  # the PE transposes can start while K/Q are still loading
                  qkvs = atpool.tile([64, 3, NCORE, TOKP], DT, tag="qkvs")
                  for t in (2, 1, 0):
                      nc.sync.dma_start(qkvs[:, t],
                                        a2a_r[:, :, t, :].rearrange("s r x -> r s x"))
                  qs = qkvs[:, 0]
                  ks = qkvs[:, 1]
                  vs = qkvs[:, 2]

                  # ---- V^T -> token-major V_aug first (PE transposes start
                  # as soon as the V slice of the A2A lands) ----
                  vtA = atpool.tile([64, NF, S], DT, tag="vtA")
                  nc.vector.tensor_copy(
                      vtA[:].rearrange("p f (s j) -> p f s j", s=NCORE),
                      vs[:, :, 0:NOWN].rearrange("p s (f j) -> p f s j", f=NF))

                  v_a = atpool.tile([128, 2 * NF, VA], DT, tag="va")
                  nc.vector.memset(v_a[:, :, DH], 1.0)
                  nc.vector.memset(v_a[:, :, DH + 1], 0.0)
                  for f in range(NF):
                      pst = big3.tile([128, 128], DTT, tag="ps3")
                      for t2 in range(2):
                          nc.tensor.transpose(
                              pst[:, 64 * t2:64 * t2 + 64],
                              vtA[:, f, 128 * t2:128 * (t2 + 1)],
                              identT[:64, :64])
                      nc.vector.tensor_copy(
                          v_a[:, 2 * f:2 * f + 2, 0:DH],
                          _ps32(pst[:].rearrange("p (t c) -> p t c", t=2)))
                  v_l = atpool.tile([NFP, VA], DT, tag="vl")
                  nc.vector.memset(v_l[:NFP, DH:DH + 1], 1.0)
                  nc.vector.memset(v_l[:NFP, DH + 1:DH + 2], 0.0)
                  pstl = big3.tile([NFP, 64], DTT, tag="ps3")
                  nc.tensor.transpose(pstl[:NFP, :64], _M(vs[:, 7, NOWN:TOKP]),
                                      identT[:64, :64])
                  nc.vector.tensor_copy(v_l[:NFP, 0:DH], _ps32(pstl[:NFP, :64]))

                  # ---- assemble K and Q frame-major ----
                  ktA = atpool.tile([64, NF, S], DT, tag="ktA")
                  nc.vector.tensor_copy(
                      ktA[:].rearrange("p f (s j) -> p f s j", s=NCORE),
                      ks[:, :, 0:NOWN].rearrange("p s (f j) -> p f s j", f=NF))
                  ktl = atpool.tile([64, NFP], DT, tag="ktl")
                  nc.vector.tensor_copy(ktl[:64, :], ks[:, 7, NOWN:TOKP])
                  qtA = atpool.tile([64, NF, TPQ], DT, tag="qtA")
                  nc.vector.tensor_scalar(
                      qtA[:, :, 0:S].rearrange("p f (s j) -> p f s j", s=NCORE),
                      qs[:, :, 0:NOWN].rearrange("p s (f j) -> p f s j", f=NF),
                      bq_sb[:64, 0:1], None, op0=ALU.add)
                  nc.vector.tensor_scalar(qtA[:, :, S], qs[:, 7, NOWN:TOK],
                                          bq_sb[:64, 0:1], None, op0=ALU.add)
                  nc.vector.memset(qtA[:, :, S + 1], 0.0)

                  # ---- attention (this head, all 11 q-frames) ----
                  otA = atpool.tile([64, NF, TPQ], DT, tag="qkvs")
                  nfq = NZ if i == n_blocks - 1 else NF
                  for fq in range(nfq):
                      kvf = _kv_frames(fq)
                      n_main = 2 * len(kvf)
                      ps_o = pspool.tile([VA, TPQ], F32, tag="psbig")
                      ps_l = big3.tile([NFP, TPQ], F32, tag="ps3")
                      nc.tensor.matmul(ps_l[:NFP], _M(ktl[:64, :]),
                                       _M(qtA[:, fq, :]), start=True, stop=True)
                      pl = ptpool.tile([NFP, TPQ], DTT, tag="pt")
                      nc.scalar.activation(pl[:NFP, :], ps_l[:NFP, :], ACTF.Exp,
                                           scale=0.125, bias=lbias[:NFP, fq:fq + 1])
                      nc.tensor.matmul(ps_o[:], _M(v_l[:NFP, :]), pl[:NFP],
                                       start=True, stop=False)
                      av_i = 0
                      for fi in kvf:
                          for t2 in range(2):
                              ps_s = big3.tile([128, TPQ], F32, tag="ps3")
                              nc.tensor.matmul(
                                  ps_s[:],
                                  ktA[:, fi, 128 * t2:128 * (t2 + 1)],
                                  _M(qtA[:, fq, :]), start=True, stop=True)
                              pt = ptpool.tile([128, TPQ], DTT, tag="pt")
                              nc.scalar.activation(pt[:], ps_s[:], ACTF.Exp, scale=0.125)
                              nc.tensor.matmul(ps_o[:], _M(v_a[:, 2 * fi + t2, :]), pt[:],
                                               start=False, stop=(av_i == n_main - 1))
                              av_i += 1
                      rc = scpool.tile([1, TPQ], F32, tag="rc")
                      nc.vector.reciprocal(rc[0:1], ps_o[DH:DH + 1, :])
                      bc = scpool.tile([64, TPQ], F32, tag="rc")
                      nc.gpsimd.partition_broadcast(bc[:64], rc[0:1, :])
                      nc.vector.tensor_mul(otA[:, fq, :], ps_o[0:DH, :], bc[:64])

                  # ---- A2A back: chunk d = my head's outputs for d's tokens ----
                  bk_send = atpool.tile([64, NCORE, TOKP], DT, tag="bks")
                  nc.vector.tensor_copy(
                      bk_send[:, :, 0:NOWN].rearrange("p d (f j) -> p d f j", f=NF),
                      otA[:, :, 0:S].rearrange("p f (d j) -> p d f j", d=NCORE))
                  nc.vector.tensor_copy(
                      bk_send[:, :, NOWN:TOK],
                      otA[:, None, :, S].broadcast_to([64, NCORE, NF]))
                  nc.vector.memset(bk_send[:, :, TOK], 0.0)
                  bk_s = dpool.tile([NCORE, 64, TOKP], DT, tag=f"bks{rr}_{i}")
                  bk_r = dpool.tile([NCORE, 64, TOKP], DT, tag=f"bkr{rr}_{i}")
                  nc.sync.dma_start(bk_s.rearrange("d r x -> r d x"), bk_send[:])
                  if skip_coll:
                      nc.sync.dma_start(bk_r[:], bk_s[:])
                  else:
                      nc.gpsimd.collective_compute(
                          "AllToAll", ALU.bypass, replica_groups=[CORE_IDS],
                          ins=[bk_s.opt()], outs=[bk_r.opt()])
                  xaT = apool.tile([128, 4, TOKP], DT, tag="qt")
                  nc.sync.dma_start(xaT[:],
                                    bk_r.rearrange("(c p2) r x -> (p2 r) c x", c=4))

                  # last block: xa tokens and register/leftover rows are dead
                  # past attention -> keep only zr own rows (cols 0:192)
                  last = i == n_blocks - 1 and rr == repeat - 1
                  tts_live = ([(0, (0, 128)), (1, (128, 64))] if last
                              else list(enumerate(TOKT)))

                  # ---- Wo + gate1*xn1 + bo' ----
                  x2g = {}
                  for tt, (r0, p_) in tts_live:
                      x2t = apool.tile([128, D], F32, tag=f"x2_{tt}")
                      pso = pspool.tile([128, D], F32, tag="psbig")
                      for k in range(4):
                          nc.tensor.matmul(pso[:p_], _M(xaT[:, k, r0:r0 + p_]),
                                           wqkvo[:, 12 + k, :], start=(k == 0), stop=(k == 3))
                      tmp = scpool.tile([128, D], F32, tag="tmpD")
                      nc.vector.tensor_add(tmp[:p_], pso[:p_], bop_sb[:p_])
                      nc.vector.tensor_add(x2t[:p_, :], tmp[:p_], g1x_t[tt][:p_, :])
                      x2g[tt] = x2t

                  # ---- mod2 ----
                  xn2 = apool.tile([128, 3, D], F32R, tag="xn1")
                  ln_mod(x2g, 3, 4, xn2, tts=tts_live)
                  xn2T = apool.tile([128, 4, TOKP], DTF, tag="xnT")
                  transpose_tok(xn2, xn2T, tts=tts_live)

                  # ---- GEGLU + FF out, streamed in 8 weight pieces ----
                  psf = {tt: big3.tile([128, D], F32, tag="ps3", name=f"psf{rr}_{i}_{tt}")
                         for tt, _ in tts_live}
                  tok_rs = ((0, 192),) if last else ((0, TOKP),)
                  for p in range(8):
                      wgp = wgpool.tile([128, 4, 512], DTF, tag="wgp")
                      nc.sync.dma_start(wgp[:], e["wg"][:, :, 512 * p:512 * (p + 1)])
                      wfp = wgpool.tile([128, 2, D], DTF, tag="wfp")
                      nc.sync.dma_start(wfp[:], e["wf"][:, 2 * p:2 * (p + 1), :])
                      for j in range(2):
                          mm = 2 * p + j
                          psa = pspool.tile([128, TOKP], F32, tag="psbig")
                          for c0, c1 in tok_rs:
                              for k in range(4):
                                  nc.tensor.matmul(psa[:, c0:c1], wgp[:, k, 256 * j:256 * j + 128],
                                                   xn2T[:, k, c0:c1], start=(k == 0), stop=(k == 3))
                          # pstab's banks are idle through the GEGLU body; using
                          # them for psg doubles the a/g matmul pipeline depth
                          psg = pstab.tile([128, TOKP], F32, tag="pstab")
                          for c0, c1 in tok_rs:
                              for k in range(4):
                                  nc.tensor.matmul(psg[:, c0:c1], wgp[:, k, 256 * j + 128:256 * j + 256],
                                                   xn2T[:, k, c0:c1], start=(k == 0), stop=(k == 3))
                          gel = scpool.tile([128, TOKP], F32, tag="gel")
                          hp = hpool.tile([128, TOKP], DTF, tag="hp")
                          for c0, c1 in tok_rs:
                              nc.scalar.activation(gel[:, c0:c1], psg[:, c0:c1], ACTF.Gelu,
                                                   bias=bgl_sb[:, 2 * mm + 1:2 * mm + 2])
                              nc.vector.scalar_tensor_tensor(hp[:, c0:c1], psa[:, c0:c1],
                                                             bgl_sb[:, 2 * mm:2 * mm + 1], gel[:, c0:c1],
                                                             op0=ALU.add, op1=ALU.mult)
                          for tt, (r0, p_) in tts_live:
                              nc.tensor.matmul(psf[tt][:p_], _M(hp[:, r0:r0 + p_]),
                                               wfp[:, j, :],
                                               start=(mm == 0), stop=(mm == 15))

                  x_new = xpool.tile([128, 3, D], F32, tag="x")
                  for tt, (r0, p_) in tts_live:
                      tmp = scpool.tile([128, D], F32, tag="tmpD")
                      nc.vector.tensor_add(tmp[:p_], psf[tt][:p_], bff_sb[:p_])
                      nc.vector.tensor_mul(x_new[:p_, tt, :], tmp[:p_],
                                           tabb[:p_, tt * 6 + 5, :])
                  x = x_new
                  if _os.environ.get("KERNEL_BLOCK_BARRIER"):
                      # optional scheduling barrier between blocks (collective
                      # ordering is data-enforced; barrier-free validated on HW)
                      tc.strict_bb_all_engine_barrier()

            # only zr own rows (token slots 0:192) survive the last block
            nc.sync.dma_start(xout_e[0:128, :], x[:, 0, :])
            nc.sync.dma_start(xout_e[128:192, :], x[:64, 1, :])
    nc.compile()
    return nc


# ----------------------------------------------------------------------
# host side
# ----------------------------------------------------------------------
def _silu(x):
    return x / (1.0 + np.exp(-x))


def _frame_of():
    """frame index of each per-core token slot (12 = pad/zero row)."""
    fr = np.full(TOKP, NF, np.int64)
    fr[:NOWN] = np.arange(NOWN) // OWN
    fr[NOWN:TOK] = np.arange(NF)
    return fr


def _host_prep(inputs, n_blocks):
    f32 = np.float32
    z = np.asarray(inputs['z'], f32)
    frames = np.asarray(inputs['frames'], f32)
    actions = np.asarray(inputs['actions'])
    ts = np.asarray(inputs['ts'])

    def patch(xx):
        b, dur, c, h, w = xx.shape
        xx = xx.reshape(b, dur, c, h // P2, P2, w // P2, P2)
        xx = xx.transpose(0, 1, 3, 5, 2, 4, 6).reshape(b, dur, (h // P2) * (w // P2), c * P2 * P2)
        return xx @ np.asarray(inputs['W_patch'], f32) + np.asarray(inputs['b_patch'], f32)

    pe = np.asarray(inputs['pe_grid'], f32)
    zt = patch(z)[0] + pe[None]
    xt = patch(frames)[0] + pe[None]
    reg = np.asarray(inputs['registers'], f32)
    aemb = np.asarray(inputs['action_emb'], f32)
    temb = np.asarray(inputs['time_emb'], f32)
    a = aemb[actions[0]]

    ft = np.zeros((NF, TPF, D), f32)
    for f in range(NZ):
        ft[f, :S] = zt[f]
        ft[f, S] = reg[0]
    for f in range(NX):
        ft[NZ + f, :S] = xt[f]
        ft[NZ + f, S] = a[f]

    cond = np.zeros((NF, D), f32)
    for f in range(NZ):
        cond[f] = temb[ts[0, f]]
    for f in range(NX):
        cond[NZ + f] = temb[0]
    sc = _silu(cond)

    blocks = []
    for i in range(n_blocks):
        m1 = sc @ np.asarray(inputs['W_mod1'][i], f32) + np.asarray(inputs['b_mod1'][i], f32)
        s1, t1 = np.split(m1, 2, -1)
        m2 = sc @ np.asarray(inputs['W_mod2'][i], f32) + np.asarray(inputs['b_mod2'][i], f32)
        s2, t2 = np.split(m2, 2, -1)
        g1 = cond @ np.asarray(inputs['W_g1'][i], f32) + np.asarray(inputs['b_g1'][i], f32)
        g2 = cond @ np.asarray(inputs['W_g2'][i], f32) + np.asarray(inputs['b_g2'][i], f32)
        bo_p = (np.asarray(inputs['b_o'][i], f32)
                + np.asarray(inputs['b_v'][i], f32) @ np.asarray(inputs['W_o'][i], f32))
        tabf = np.zeros((NFP + 1, 6, D), f32)
        tabf[:NF] = np.stack([1.0 + s1, t1, g1, 1.0 + s2, t2, g2], 1)
        # token-broadcast form: row p of group tt gets frame fr(tt*128+p)
        fr = _frame_of()
        frp = np.full(3 * 128, NFP, np.int64)
        frp[:TOKP] = np.minimum(fr, NFP)
        tabb = tabf[frp.reshape(3, 128)]            # [3, 128, 6, D]
        tabb = np.ascontiguousarray(
            tabb.transpose(1, 0, 2, 3).reshape(128, 18, D)).astype(np.float16)

        def chunk(w, kparts):
            K, N = w.shape
            return np.ascontiguousarray(
                np.asarray(w, f32).reshape(kparts, 128, N).swapaxes(0, 1))

        wq = chunk(np.asarray(inputs['W_q'][i]), 4)
        wk = chunk(np.asarray(inputs['W_k'][i]), 4)
        wv = chunk(np.asarray(inputs['W_v'][i]), 4)
        wo = chunk(np.asarray(inputs['W_o'][i]), 4)
        wqkvo = np.concatenate([wq, wk, wv, wo], 1)

        # interleave a/g columns of W_geglu so each 256-col group is (a_mm|g_mm)
        wg = chunk(np.asarray(inputs['W_geglu'][i]), 4)        # [128, 4, 4096]
        wg4 = wg.reshape(128, 4, 2, 16, 128)                   # [., ., a/g, mm, col]
        wg_i = np.ascontiguousarray(
            wg4.transpose(0, 1, 3, 2, 4).reshape(128, 4, 4096))
        bg = np.asarray(inputs['b_geglu'][i], f32).reshape(2, 16, 128)
        bgl = np.ascontiguousarray(
            bg.transpose(2, 1, 0).reshape(128, 32))            # [128, 32] cols (2mm, 2mm+1)

        wf_i = chunk(np.asarray(inputs['W_ffout'][i]), 16)
        if ATT_BF:
            import ml_dtypes
            wqkvo = wqkvo.astype(ml_dtypes.bfloat16)
        if FFN_F16:
            wg_i = wg_i.astype(np.float16)
            wf_i = wf_i.astype(np.float16)
        blocks.append(dict(
            wqkvo=wqkvo,
            wg=wg_i,
            wf=wf_i,
            bq=np.asarray(inputs['b_q'][i], f32),   # sliced per core below
            bgl=bgl,
            bop=np.tile(bo_p[None], (128, 1)).astype(f32),
            bff=np.tile(np.asarray(inputs['b_ffout'][i], f32)[None], (128, 1)),
            tabb=tabb,
        ))
    return ft, blocks


def kernel(**inputs):
    import os
    n_blocks = int(os.environ.get("KERNEL_NBLOCKS", NB))
    ft, blocks = _host_prep(inputs, n_blocks)

    lb = np.full((NFP, NFP), -30.0, np.float32)
    for fq in range(NF):
        for kf in _kv_frames(fq):
            lb[kf, fq] = 0.0

    in_maps = []
    for c in range(NCORE):
        x0p = np.zeros((TOKP, D), np.float32)
        for f in range(NF):
            x0p[f * OWN:(f + 1) * OWN] = ft[f, OWN * c:OWN * (c + 1)]
            x0p[NOWN + f] = ft[f, S]
        m = {"x0": x0p, "lbias": lb}
        for i in range(n_blocks):
            for k, v in blocks[i].items():
                if k == "bq":
                    v = np.ascontiguousarray(v[64 * c:64 * (c + 1)].reshape(64, 1))
                m[f"{k}{i}"] = v
        in_maps.append(m)

    repeat = int(os.environ.get("KERNEL_REPEAT", 1))
    key = (n_blocks, repeat)
    if key not in _CACHE:
        _CACHE[key] = _build(n_blocks, repeat)
    nc = _CACHE[key]
    trace = bool(os.environ.get("KERNEL_TRACE"))
    res = run_bass_kernel_spmd(nc, in_maps, CORE_IDS, trace=trace)
    global LAST_RESULT
    LAST_RESULT = res

    out = np.zeros((NF, TPF, D), np.float32)
    for c in range(NCORE):
        xo = res.results[c]["xout"]
        for f in range(NF):
            out[f, OWN * c:OWN * (c + 1)] = xo[f * OWN:(f + 1) * OWN]
    x0 = res.results[0]["xout"]
    for f in range(NF):
        out[f, S] = x0[NOWN + f]

    f32 = np.float32
    zr = out[:NZ, :S]
    y = zr @ np.asarray(inputs['W_unpatch'], f32) + np.asarray(inputs['b_unpatch'], f32)
    y = y.reshape(1, NZ, HH // P2, WW // P2, C, P2, P2)
    y = y.transpose(0, 1, 4, 2, 5, 3, 6).reshape(1, NZ, C, HH, WW)
    return np.ascontiguousarray(y.astype(np.float32))

